# revision 21
# baseline (speedup 1.0000x reference)
"""MoE FFN (8 experts, top-2, SwiGLU) Trainium2 kernel.

Expert-parallel: core e holds expert e's weights. The router (logits,
softmax, top-2 selection AND combine weights) is computed on the host in
float64; tokens are dispatched to the cores owning their top-2 experts,
padded to an adaptive per-expert capacity CAP (max expert load, rounded
up to a multiple of 4). Each core runs the SwiGLU FFN over its CAP
tokens entirely in bf16 (PE rate matches fp32r while DMA/SBUF halve and
LDWEIGHTS gets FWL), scales by the per-token combine weight, and writes
y^T [D, CAP] in fp32. The host scatter-adds the partials back into
token order.

Device structure:
  phase A (x-chunk outer):  hT[h, tok] = silu(x@wg)^T * (x@wv)^T
      stationary = wg/wv d-tiles [128, 128], moving = x^T token chunks.
  phase B (d-tile outer):   yT[d, tok] = wo^T @ hT, * w[tok]
      stationary = wo h-tiles [128, 128], moving = hT token chunks; the
      combine weight is applied as a DVE multiply against a
      partition-broadcast copy of w, and y^T stores are one DMA per
      d-tile.

DMA plan: two HWDGE rings drain concurrently. The scalar ring carries
the first x chunk + w broadcast (+ y stores later); the sync ring
carries the wg/wv stream (one 512 KB DMA per h-tile, consumed ~3.4 us
apart), the remaining x chunks, then all of wo. A burst of junk
matmuls on a memset tile ramps the HAM clock gate while the first
loads land.

Self-contained: shapes hardcoded for x[2,2048,1024], 8 experts,
d_expert=2048, top-2; capacity adapts to the routed load at first call
(kernel compiled per distinct CAP and cached).
"""

import math
from contextlib import ExitStack

import ml_dtypes
import numpy as np

import concourse.bass as bass
import concourse.mybir as mybir
import concourse.tile as tile
from concourse import bacc
from concourse.bass_utils import run_bass_kernel_spmd

# ---- problem constants --------------------------------------------------
B, T, D = 2, 2048, 1024
N_TOK = B * T          # 4096 tokens
E = 8                  # experts == cores
H = 2048               # expert hidden dim
TOP_K = 2
P = 128
ND = D // P            # 8  d-tiles (contraction tiles of d_model)
NH = H // P            # 16 h-tiles
NWO = 4                # wo DMA blocks (4 h-tiles each)

CAP_LIMIT = 1280       # SBUF budget bound; beyond this, dispatch in rounds
N_WARM = 80            # PE warm-up matmuls (ramp HAM + bridge the DMA ramp)

FP = mybir.dt.float32
BF = mybir.dt.bfloat16
AF = mybir.ActivationFunctionType
OP = mybir.AluOpType
BF_NP = ml_dtypes.bfloat16


def _chunks(cap):
    """Token-chunk schedule: a narrow first chunk so phase A can start on
    a small x load, then 512-wide chunks (the PSUM bank limit)."""
    first = min(320, cap)
    out = [(0, first)]
    s = first
    while s < cap:
        out.append((s, min(512, cap - s)))
        s += out[-1][1]
    return out


def _emit(nc, tc, ctx, cap, xt_d, wgv_d, wo_d, w_d, y_d):
    chunks = _chunks(cap)
    const = ctx.enter_context(tc.tile_pool(name="const", bufs=1))
    x_pool = ctx.enter_context(tc.tile_pool(name="x", bufs=1))
    wgv_pool = ctx.enter_context(tc.tile_pool(name="wgv", bufs=1))
    wo_pool = ctx.enter_context(tc.tile_pool(name="wo", bufs=1))
    ht_pool = ctx.enter_context(tc.tile_pool(name="ht", bufs=1))
    act_pool = ctx.enter_context(tc.tile_pool(name="act", bufs=3))
    yst_pool = ctx.enter_context(tc.tile_pool(name="yst", bufs=2))

    # x arrives host-pre-tiled per chunk: xt_d[p, ND*cs + dt*cw + c], so
    # each chunk load is one contiguous 2*ND*cw-byte run per partition.
    xc = []
    for ci, (cs, cw) in enumerate(chunks):
        xtile = x_pool.tile([P, ND, cw], BF, tag=f"x{ci}", name=f"xc{ci}")
        xc.append(xtile)

    # sync ring, in consumption order: the first x chunk and wg/wv of
    # h-tile 0 land first (split into halves so the d0-3 matmuls of the
    # first PSUM group can start on a 0.5 MB footprint), then the rest
    # of the wg/wv stream, the remaining x chunks, then wo.
    W2 = 2 * ND * P
    cw0 = chunks[0][1]
    wgvt = [
        wgv_pool.tile([P, W2], BF, tag=f"wgv{hk}", name=f"wgv{hk}")
        for hk in range(NH)
    ]
    x0_ap = xt_d.ap()[:, 0:ND * cw0].rearrange("p (dt c) -> p dt c", dt=ND)
    nc.sync.dma_start(out=xc[0][:, 0:ND // 2, :], in_=x0_ap[:, 0:ND // 2, :])
    nc.sync.dma_start(out=wgvt[0][:, :ND * P], in_=wgv_d.ap()[:, :ND * P])
    nc.sync.dma_start(out=xc[0][:, ND // 2:, :], in_=x0_ap[:, ND // 2:, :])
    nc.sync.dma_start(out=wgvt[0][:, ND * P:], in_=wgv_d.ap()[:, ND * P:W2])
    for hk in range(1, NH):
        nc.sync.dma_start(
            out=wgvt[hk][:], in_=wgv_d.ap()[:, hk * W2:(hk + 1) * W2]
        )

    # scalar ring: w broadcast (needed at phase B); y stores ride this
    # ring later.
    wbc = const.tile([P, cap], FP)
    nc.scalar.dma_start(out=wbc[:], in_=w_d.ap().partition_broadcast(P))
    for ci, (cs, cw) in enumerate(chunks[1:], start=1):
        nc.sync.dma_start(
            out=xc[ci][:],
            in_=xt_d.ap()[:, ND * cs:ND * (cs + cw)]
            .rearrange("p (dt c) -> p dt c", dt=ND),
        )
    wo_ap = wo_d.ap().rearrange("(b j p) c -> p b j c", b=NWO, j=NH // NWO)
    wot = []
    for b in range(NWO):
        wob = wo_pool.tile([P, NH // NWO, D], BF, tag=f"wo{b}", name=f"wo{b}")
        nc.sync.dma_start(out=wob[:], in_=wo_ap[:, b, :, :])
        wot.append(wob)

    ht = [
        ht_pool.tile([P, cap], BF, tag=f"h{hk}", name=f"ht{hk}") for hk in range(NH)
    ]

    # ---- PE warm-up: ramp the HAM clock gate while the first loads land
    ones = const.tile([P, P], BF)
    nc.vector.memset(ones[:], 1.0)
    with ExitStack() as wctx:
        ps_w = wctx.enter_context(tc.tile_pool(name="psw", bufs=1, space="PSUM"))
        warm = ps_w.tile([E, P], FP, name="warm", tag="warm")
        for _ in range(N_WARM):
            nc.tensor.matmul(
                warm[:], lhsT=ones[:, :E], rhs=ones[:], start=True, stop=True
            )

    # ---- phase A: hT[h, tok] = silu(x@wg)^T * (x@wv)^T ------------------
    with ExitStack() as actx:
        ps_g = actx.enter_context(tc.tile_pool(name="psg", bufs=2, space="PSUM"))
        ps_v = actx.enter_context(tc.tile_pool(name="psv", bufs=2, space="PSUM"))
        for ci, (cs, cw) in enumerate(chunks):
            for hk in range(NH):
                pg = ps_g.tile([P, 512], FP)
                pv = ps_v.tile([P, 512], FP)
                for d in range(ND):
                    nc.tensor.matmul(
                        pg[:, :cw],
                        lhsT=wgvt[hk][:, d * P:(d + 1) * P],
                        rhs=xc[ci][:, d, :],
                        start=(d == 0),
                        stop=(d == ND - 1),
                    )
                for d in range(ND):
                    nc.tensor.matmul(
                        pv[:, :cw],
                        lhsT=wgvt[hk][:, ND * P + d * P:ND * P + (d + 1) * P],
                        rhs=xc[ci][:, d, :],
                        start=(d == 0),
                        stop=(d == ND - 1),
                    )
                sg = act_pool.tile([P, 512], FP, tag="sg")
                nc.scalar.activation(sg[:, :cw], pg[:, :cw], AF.Silu)
                nc.vector.tensor_tensor(
                    ht[hk][:, cs:cs + cw], pv[:, :cw], sg[:, :cw], op=OP.mult
                )

    # ---- phase B: yT[d, tok] = (wo^T @ hT) * w --------------------------
    with ExitStack() as bctx:
        # bufs=1: three chunk banks, disjoint from phase A's four — avoids
        # an A->B boundary stall on A's last PSUM drains; chunk mults
        # complete before the next d-tile's group needs the bank.
        ps_y = bctx.enter_context(tc.tile_pool(name="psy", bufs=1, space="PSUM"))
        for dt in range(ND):
            pys = [
                ps_y.tile([P, 512], FP, tag=f"c{ci}", name=f"py{dt}c{ci}")
                for ci in range(len(chunks))
            ]
            ysb = yst_pool.tile([P, cap], BF, tag="y", name=f"y{dt}")
            if dt < ND - 1:
                # interleave chunks per hk: one wo weight-load feeds all
                # three chunk matmuls
                for hk in range(NH):
                    for ci, (cs, cw) in enumerate(chunks):
                        nc.tensor.matmul(
                            pys[ci][:, :cw],
                            lhsT=wot[hk // NWO][:, hk % NWO, dt * P:(dt + 1) * P],
                            rhs=ht[hk][:, cs:cs + cw],
                            start=(hk == 0),
                            stop=(hk == NH - 1),
                        )
                for ci, (cs, cw) in enumerate(chunks):
                    nc.vector.tensor_tensor(
                        ysb[:, cs:cs + cw], pys[ci][:, :cw], wbc[:, cs:cs + cw],
                        op=OP.mult,
                    )
                nc.scalar.dma_start(
                    out=y_d.ap()[dt * P:(dt + 1) * P, :], in_=ysb[:]
                )
            else:
                # final d-tile: chunk-outer so earlier chunks scale + store
                # while later chunks are still accumulating; the kernel's
                # tail is one narrow store, not the whole row block
                for ci, (cs, cw) in enumerate(chunks):
                    for hk in range(NH):
                        nc.tensor.matmul(
                            pys[ci][:, :cw],
                            lhsT=wot[hk // NWO][:, hk % NWO, dt * P:(dt + 1) * P],
                            rhs=ht[hk][:, cs:cs + cw],
                            start=(hk == 0),
                            stop=(hk == NH - 1),
                        )
                    nc.vector.tensor_tensor(
                        ysb[:, cs:cs + cw], pys[ci][:, :cw], wbc[:, cs:cs + cw],
                        op=OP.mult,
                    )
                    nc.scalar.dma_start(
                        out=y_d.ap()[dt * P:(dt + 1) * P, cs:cs + cw],
                        in_=ysb[:, cs:cs + cw],
                    )


def _build(cap):
    nc = bacc.Bacc("TRN2", target_bir_lowering=False, debug=False)
    xt_d = nc.dram_tensor("xt", [P, ND * cap], BF, kind="ExternalInput")
    wgv_d = nc.dram_tensor("wgv", [P, NH * 2 * ND * P], BF, kind="ExternalInput")
    wo_d = nc.dram_tensor("wo", [H, D], BF, kind="ExternalInput")
    w_d = nc.dram_tensor("w", [1, cap], FP, kind="ExternalInput")
    y_d = nc.dram_tensor("y", [D, cap], BF, kind="ExternalOutput")
    with tile.TileContext(nc) as tc:
        with ExitStack() as ctx:
            _emit(nc, tc, ctx, cap, xt_d, wgv_d, wo_d, w_d, y_d)
    nc.compile()
    return nc


_NCS = {}


def _get_nc(cap):
    if cap not in _NCS:
        _NCS[cap] = _build(cap)
    return _NCS[cap]


def _route(xf, gate_w, expert_bias):
    """Host router in float64: top-2 selection + normalized combine weights."""
    logits = xf.astype(np.float64) @ gate_w.astype(np.float64) + expert_bias.astype(
        np.float64
    )
    m = logits.max(axis=-1, keepdims=True)
    p = np.exp(logits - m)
    p /= p.sum(axis=-1, keepdims=True)
    # ties -> lower index first, matching jax.lax.top_k
    order = np.argsort(-p, axis=-1, kind="stable")[:, :TOP_K]
    rw = np.take_along_axis(p, order, axis=-1)
    rw = rw / (rw.sum(axis=-1, keepdims=True) + 1e-8)
    return order, rw


def _tile_wgv(wg, wv):
    """Two [D, H] fp32 -> [128, NH*2*ND*128] bf16, [p, hk, {g,v}, dt, h]."""
    def t(w):
        return w.astype(BF_NP).reshape(ND, P, NH, P).transpose(1, 2, 0, 3)

    return np.ascontiguousarray(
        np.stack([t(wg), t(wv)], axis=2).reshape(P, NH * 2 * ND * P)
    )


def kernel(x, gate_w, expert_bias, w_gate, w_value, w_out, _trace=False):
    x = np.asarray(x, dtype=np.float32)
    gate_w = np.asarray(gate_w, dtype=np.float32)
    expert_bias = np.asarray(expert_bias, dtype=np.float32)
    w_gate = np.asarray(w_gate, dtype=np.float32)
    w_value = np.asarray(w_value, dtype=np.float32)
    w_out = np.asarray(w_out, dtype=np.float32)

    xf = np.ascontiguousarray(x.reshape(N_TOK, D))
    order, rw = _route(xf, gate_w, expert_bias)
    idx = [np.flatnonzero((order == e).any(axis=-1)) for e in range(E)]
    max_load = max(1, max(len(i) for i in idx))
    cap = min(CAP_LIMIT, -4 * (-max_load // 4))
    n_rounds = max(1, math.ceil(max_load / cap))

    nc = _get_nc(cap)
    wgv_t = [_tile_wgv(w_gate[e], w_value[e]) for e in range(E)]
    wo_t = [np.ascontiguousarray(w_out[e].astype(BF_NP)) for e in range(E)]
    # per-token combine weight of each token for expert e
    w_of = [
        np.where(
            order[:, 0] == e,
            rw[:, 0],
            np.where(order[:, 1] == e, rw[:, 1], 0.0),
        ).astype(np.float32)
        for e in range(E)
    ]

    out = np.zeros((N_TOK, D), dtype=np.float32)
    last = None
    for r in range(n_rounds):
        in_maps = []
        for e in range(E):
            ids = idx[e][r * cap:(r + 1) * cap]
            ids_p = np.zeros(cap, dtype=np.int64)
            ids_p[: len(ids)] = ids
            # [P, ND*cap] with [p, chunk-major (dt, c)] layout so each
            # chunk load is one contiguous run per partition
            xT = xf[ids_p].T.astype(BF_NP).reshape(ND, P, cap)
            xt = np.concatenate(
                [
                    np.ascontiguousarray(
                        xT[:, :, cs:cs + cw].transpose(1, 0, 2)
                    ).reshape(P, ND * cw)
                    for cs, cw in _chunks(cap)
                ],
                axis=1,
            )
            w_pad = np.zeros((1, cap), dtype=np.float32)
            w_pad[0, : len(ids)] = w_of[e][ids]
            in_maps.append({
                "xt": xt,
                "wgv": wgv_t[e],
                "wo": wo_t[e],
                "w": w_pad,
            })
        res = run_bass_kernel_spmd(
            nc, in_maps, core_ids=list(range(E)),
            trace=bool(_trace), trace_cores=list(range(E)) if _trace else None,
        )
        last = res
        for e in range(E):
            ids = idx[e][r * cap:(r + 1) * cap]
            if len(ids):
                out[ids] += res.results[e]["y"][:, : len(ids)].T.astype(np.float32)
    if _trace:
        kernel.last_results = last
    return out.reshape(B, T, D)


# revision 22
# speedup vs baseline: 1.0124x; 1.0124x over previous
"""MoE FFN (8 experts, top-2, SwiGLU) Trainium2 kernel.

Expert-parallel: core e holds expert e's weights. The router (logits,
softmax, top-2 selection AND combine weights) is computed on the host in
float64; tokens are dispatched to the cores owning their top-2 experts,
padded to an adaptive per-expert capacity CAP (max expert load, rounded
up to a multiple of 4). Each core runs the SwiGLU FFN over its CAP
tokens entirely in bf16 (PE rate matches fp32r while DMA/SBUF halve and
LDWEIGHTS gets FWL), scales by the per-token combine weight, and writes
y^T [D, CAP] in fp32. The host scatter-adds the partials back into
token order.

Device structure:
  phase A (x-chunk outer):  hT[h, tok] = silu(x@wg)^T * (x@wv)^T
      stationary = wg/wv d-tiles [128, 128], moving = x^T token chunks.
  phase B (d-tile outer):   yT[d, tok] = wo^T @ hT, * w[tok]
      stationary = wo h-tiles [128, 128], moving = hT token chunks; the
      combine weight is applied as a DVE multiply against a
      partition-broadcast copy of w, and y^T stores are one DMA per
      d-tile.

DMA plan: two HWDGE rings drain concurrently. The scalar ring carries
the first x chunk + w broadcast (+ y stores later); the sync ring
carries the wg/wv stream (one 512 KB DMA per h-tile, consumed ~3.4 us
apart), the remaining x chunks, then all of wo. A burst of junk
matmuls on a memset tile ramps the HAM clock gate while the first
loads land.

Self-contained: shapes hardcoded for x[2,2048,1024], 8 experts,
d_expert=2048, top-2; capacity adapts to the routed load at first call
(kernel compiled per distinct CAP and cached).
"""

import math
from contextlib import ExitStack

import ml_dtypes
import numpy as np

import concourse.bass as bass
import concourse.mybir as mybir
import concourse.tile as tile
from concourse import bacc
from concourse.bass_utils import run_bass_kernel_spmd

# ---- problem constants --------------------------------------------------
B, T, D = 2, 2048, 1024
N_TOK = B * T          # 4096 tokens
E = 8                  # experts == cores
H = 2048               # expert hidden dim
TOP_K = 2
P = 128
ND = D // P            # 8  d-tiles (contraction tiles of d_model)
NH = H // P            # 16 h-tiles
NWO = 4                # wo DMA blocks (4 h-tiles each)

CAP_LIMIT = 1280       # SBUF budget bound; beyond this, dispatch in rounds
N_WARM = 80            # PE warm-up matmuls (ramp HAM + bridge the DMA ramp)

FP = mybir.dt.float32
BF = mybir.dt.bfloat16
AF = mybir.ActivationFunctionType
OP = mybir.AluOpType
BF_NP = ml_dtypes.bfloat16


def _chunks(cap):
    """Token-chunk schedule: a narrow first chunk so phase A can start on
    a small x load, then 512-wide chunks (the PSUM bank limit)."""
    first = min(320, cap)
    out = [(0, first)]
    s = first
    while s < cap:
        out.append((s, min(512, cap - s)))
        s += out[-1][1]
    return out


def _emit(nc, tc, ctx, cap, xt_d, wgv_d, wo_d, w_d, y_d):
    chunks = _chunks(cap)
    const = ctx.enter_context(tc.tile_pool(name="const", bufs=1))
    x_pool = ctx.enter_context(tc.tile_pool(name="x", bufs=1))
    wgv_pool = ctx.enter_context(tc.tile_pool(name="wgv", bufs=1))
    wo_pool = ctx.enter_context(tc.tile_pool(name="wo", bufs=1))
    ht_pool = ctx.enter_context(tc.tile_pool(name="ht", bufs=1))
    act_pool = ctx.enter_context(tc.tile_pool(name="act", bufs=3))
    yst_pool = ctx.enter_context(tc.tile_pool(name="yst", bufs=2))

    # x arrives host-pre-tiled per chunk: xt_d[p, ND*cs + dt*cw + c], so
    # each chunk load is one contiguous 2*ND*cw-byte run per partition.
    xc = []
    for ci, (cs, cw) in enumerate(chunks):
        xtile = x_pool.tile([P, ND, cw], BF, tag=f"x{ci}", name=f"xc{ci}")
        xc.append(xtile)

    # sync ring, in consumption order: the first x chunk and wg/wv of
    # h-tile 0 land first (split into halves so the d0-3 matmuls of the
    # first PSUM group can start on a 0.5 MB footprint), then the rest
    # of the wg/wv stream, the remaining x chunks, then wo.
    W2 = 2 * ND * P
    cw0 = chunks[0][1]
    wgvt = [
        wgv_pool.tile([P, W2], BF, tag=f"wgv{hk}", name=f"wgv{hk}")
        for hk in range(NH)
    ]
    x0_ap = xt_d.ap()[:, 0:ND * cw0].rearrange("p (dt c) -> p dt c", dt=ND)
    nc.sync.dma_start(out=xc[0][:, 0:ND // 2, :], in_=x0_ap[:, 0:ND // 2, :])
    nc.sync.dma_start(out=wgvt[0][:, :ND * P], in_=wgv_d.ap()[:, :ND * P])
    nc.sync.dma_start(out=xc[0][:, ND // 2:, :], in_=x0_ap[:, ND // 2:, :])
    nc.sync.dma_start(out=wgvt[0][:, ND * P:], in_=wgv_d.ap()[:, ND * P:W2])
    for hk in range(1, NH):
        nc.sync.dma_start(
            out=wgvt[hk][:], in_=wgv_d.ap()[:, hk * W2:(hk + 1) * W2]
        )

    # scalar ring: w broadcast (needed at phase B); y stores ride this
    # ring later.
    wbc = const.tile([P, cap], FP)
    nc.scalar.dma_start(out=wbc[:], in_=w_d.ap().partition_broadcast(P))
    for ci, (cs, cw) in enumerate(chunks[1:], start=1):
        nc.sync.dma_start(
            out=xc[ci][:],
            in_=xt_d.ap()[:, ND * cs:ND * (cs + cw)]
            .rearrange("p (dt c) -> p dt c", dt=ND),
        )
    wo_ap = wo_d.ap().rearrange("(b j p) c -> p b j c", b=NWO, j=NH // NWO)
    wot = []
    for b in range(NWO):
        wob = wo_pool.tile([P, NH // NWO, D], BF, tag=f"wo{b}", name=f"wo{b}")
        nc.sync.dma_start(out=wob[:], in_=wo_ap[:, b, :, :])
        wot.append(wob)

    ht = [
        ht_pool.tile([P, cap], BF, tag=f"h{hk}", name=f"ht{hk}") for hk in range(NH)
    ]

    # ---- PE warm-up: ramp the HAM clock gate while the first loads land
    ones = const.tile([P, P], BF)
    nc.vector.memset(ones[:], 1.0)
    with ExitStack() as wctx:
        ps_w = wctx.enter_context(tc.tile_pool(name="psw", bufs=1, space="PSUM"))
        warm = ps_w.tile([E, P], FP, name="warm", tag="warm")
        for _ in range(N_WARM):
            nc.tensor.matmul(
                warm[:], lhsT=ones[:, :E], rhs=ones[:], start=True, stop=True
            )

    # ---- phase A: hT[h, tok] = silu(x@wg)^T * (x@wv)^T ------------------
    with ExitStack() as actx:
        ps_g = actx.enter_context(tc.tile_pool(name="psg", bufs=2, space="PSUM"))
        ps_v = actx.enter_context(tc.tile_pool(name="psv", bufs=2, space="PSUM"))
        for ci, (cs, cw) in enumerate(chunks):
            for hk in range(NH):
                pg = ps_g.tile([P, 512], FP)
                pv = ps_v.tile([P, 512], FP)
                for d in range(ND):
                    nc.tensor.matmul(
                        pg[:, :cw],
                        lhsT=wgvt[hk][:, d * P:(d + 1) * P],
                        rhs=xc[ci][:, d, :],
                        start=(d == 0),
                        stop=(d == ND - 1),
                    )
                for d in range(ND):
                    nc.tensor.matmul(
                        pv[:, :cw],
                        lhsT=wgvt[hk][:, ND * P + d * P:ND * P + (d + 1) * P],
                        rhs=xc[ci][:, d, :],
                        start=(d == 0),
                        stop=(d == ND - 1),
                    )
                sg = act_pool.tile([P, 512], FP, tag="sg")
                nc.scalar.activation(sg[:, :cw], pg[:, :cw], AF.Silu)
                nc.vector.tensor_tensor(
                    ht[hk][:, cs:cs + cw], pv[:, :cw], sg[:, :cw], op=OP.mult
                )

    # ---- phase B: yT[d, tok] = (wo^T @ hT) * w --------------------------
    with ExitStack() as bctx:
        ps_y = bctx.enter_context(tc.tile_pool(name="psy", bufs=2, space="PSUM"))
        for dt in range(ND):
            pys = [
                ps_y.tile([P, 512], FP, tag=f"c{ci}", name=f"py{dt}c{ci}")
                for ci in range(len(chunks))
            ]
            ysb = yst_pool.tile([P, cap], BF, tag="y", name=f"y{dt}")
            if dt < ND - 1:
                # interleave chunks per hk: one wo weight-load feeds all
                # three chunk matmuls
                for hk in range(NH):
                    for ci, (cs, cw) in enumerate(chunks):
                        nc.tensor.matmul(
                            pys[ci][:, :cw],
                            lhsT=wot[hk // NWO][:, hk % NWO, dt * P:(dt + 1) * P],
                            rhs=ht[hk][:, cs:cs + cw],
                            start=(hk == 0),
                            stop=(hk == NH - 1),
                        )
                for ci, (cs, cw) in enumerate(chunks):
                    nc.vector.tensor_tensor(
                        ysb[:, cs:cs + cw], pys[ci][:, :cw], wbc[:, cs:cs + cw],
                        op=OP.mult,
                    )
                nc.scalar.dma_start(
                    out=y_d.ap()[dt * P:(dt + 1) * P, :], in_=ysb[:]
                )
            else:
                # final d-tile: chunk-outer so earlier chunks scale + store
                # while later chunks are still accumulating; the kernel's
                # tail is one narrow store, not the whole row block
                for ci, (cs, cw) in enumerate(chunks):
                    for hk in range(NH):
                        nc.tensor.matmul(
                            pys[ci][:, :cw],
                            lhsT=wot[hk // NWO][:, hk % NWO, dt * P:(dt + 1) * P],
                            rhs=ht[hk][:, cs:cs + cw],
                            start=(hk == 0),
                            stop=(hk == NH - 1),
                        )
                    nc.vector.tensor_tensor(
                        ysb[:, cs:cs + cw], pys[ci][:, :cw], wbc[:, cs:cs + cw],
                        op=OP.mult,
                    )
                    nc.scalar.dma_start(
                        out=y_d.ap()[dt * P:(dt + 1) * P, cs:cs + cw],
                        in_=ysb[:, cs:cs + cw],
                    )


def _build(cap):
    nc = bacc.Bacc("TRN2", target_bir_lowering=False, debug=False)
    xt_d = nc.dram_tensor("xt", [P, ND * cap], BF, kind="ExternalInput")
    wgv_d = nc.dram_tensor("wgv", [P, NH * 2 * ND * P], BF, kind="ExternalInput")
    wo_d = nc.dram_tensor("wo", [H, D], BF, kind="ExternalInput")
    w_d = nc.dram_tensor("w", [1, cap], FP, kind="ExternalInput")
    y_d = nc.dram_tensor("y", [D, cap], BF, kind="ExternalOutput")
    with tile.TileContext(nc) as tc:
        with ExitStack() as ctx:
            _emit(nc, tc, ctx, cap, xt_d, wgv_d, wo_d, w_d, y_d)
    nc.compile()
    return nc


_NCS = {}


def _get_nc(cap):
    if cap not in _NCS:
        _NCS[cap] = _build(cap)
    return _NCS[cap]


def _route(xf, gate_w, expert_bias):
    """Host router in float64: top-2 selection + normalized combine weights."""
    logits = xf.astype(np.float64) @ gate_w.astype(np.float64) + expert_bias.astype(
        np.float64
    )
    m = logits.max(axis=-1, keepdims=True)
    p = np.exp(logits - m)
    p /= p.sum(axis=-1, keepdims=True)
    # ties -> lower index first, matching jax.lax.top_k
    order = np.argsort(-p, axis=-1, kind="stable")[:, :TOP_K]
    rw = np.take_along_axis(p, order, axis=-1)
    rw = rw / (rw.sum(axis=-1, keepdims=True) + 1e-8)
    return order, rw


def _tile_wgv(wg, wv):
    """Two [D, H] fp32 -> [128, NH*2*ND*128] bf16, [p, hk, {g,v}, dt, h]."""
    def t(w):
        return w.astype(BF_NP).reshape(ND, P, NH, P).transpose(1, 2, 0, 3)

    return np.ascontiguousarray(
        np.stack([t(wg), t(wv)], axis=2).reshape(P, NH * 2 * ND * P)
    )


def kernel(x, gate_w, expert_bias, w_gate, w_value, w_out, _trace=False):
    x = np.asarray(x, dtype=np.float32)
    gate_w = np.asarray(gate_w, dtype=np.float32)
    expert_bias = np.asarray(expert_bias, dtype=np.float32)
    w_gate = np.asarray(w_gate, dtype=np.float32)
    w_value = np.asarray(w_value, dtype=np.float32)
    w_out = np.asarray(w_out, dtype=np.float32)

    xf = np.ascontiguousarray(x.reshape(N_TOK, D))
    order, rw = _route(xf, gate_w, expert_bias)
    idx = [np.flatnonzero((order == e).any(axis=-1)) for e in range(E)]
    max_load = max(1, max(len(i) for i in idx))
    cap = min(CAP_LIMIT, -4 * (-max_load // 4))
    n_rounds = max(1, math.ceil(max_load / cap))

    nc = _get_nc(cap)
    wgv_t = [_tile_wgv(w_gate[e], w_value[e]) for e in range(E)]
    wo_t = [np.ascontiguousarray(w_out[e].astype(BF_NP)) for e in range(E)]
    # per-token combine weight of each token for expert e
    w_of = [
        np.where(
            order[:, 0] == e,
            rw[:, 0],
            np.where(order[:, 1] == e, rw[:, 1], 0.0),
        ).astype(np.float32)
        for e in range(E)
    ]

    out = np.zeros((N_TOK, D), dtype=np.float32)
    last = None
    for r in range(n_rounds):
        in_maps = []
        for e in range(E):
            ids = idx[e][r * cap:(r + 1) * cap]
            ids_p = np.zeros(cap, dtype=np.int64)
            ids_p[: len(ids)] = ids
            # [P, ND*cap] with [p, chunk-major (dt, c)] layout so each
            # chunk load is one contiguous run per partition
            xT = xf[ids_p].T.astype(BF_NP).reshape(ND, P, cap)
            xt = np.concatenate(
                [
                    np.ascontiguousarray(
                        xT[:, :, cs:cs + cw].transpose(1, 0, 2)
                    ).reshape(P, ND * cw)
                    for cs, cw in _chunks(cap)
                ],
                axis=1,
            )
            w_pad = np.zeros((1, cap), dtype=np.float32)
            w_pad[0, : len(ids)] = w_of[e][ids]
            in_maps.append({
                "xt": xt,
                "wgv": wgv_t[e],
                "wo": wo_t[e],
                "w": w_pad,
            })
        res = run_bass_kernel_spmd(
            nc, in_maps, core_ids=list(range(E)),
            trace=bool(_trace), trace_cores=list(range(E)) if _trace else None,
        )
        last = res
        for e in range(E):
            ids = idx[e][r * cap:(r + 1) * cap]
            if len(ids):
                out[ids] += res.results[e]["y"][:, : len(ids)].T.astype(np.float32)
    if _trace:
        kernel.last_results = last
    return out.reshape(B, T, D)
